# revision 13
# baseline (speedup 1.0000x reference)
"""Trainium2 Bass kernel for the GRU session decoder (nn_Decoder).

Strategy: data-parallel over batch B=64 across 8 NeuronCores (8 rows each).
Each core runs the full pipeline for its batch slice with zero collectives:
  1. ses = tanh(lin1(ses_encoding))            (transposed layouts throughout)
  2. GI = emb @ Wih.T + bih                    (batched over all T)
  3. ghses = ses @ Whh[:, H:].T + bhh          (constant over T)
  4. 50-step GRU recurrence (first-half gates only; weights stream via PE)
  5. second-half gates batched over T -> h_full
  6. HidO = h_full @ lin2.T + lin2_b + emb
  7. logits = HidO @ outW.T  (bf16 matmul, fp32 psum) + fused exp-sum for
     log-softmax denominators
Host side: input transposes/gather, output assembly, tiny target-logit sum.

Row ordering everywhere: r = t*8 + b_local  (T-major), rows per core R=400.
Gate row permutation: [r1, z1, n1, r2, z2, n2] blocks of 1024 rows each,
so in m-tile units (128 rows): 0:8=r1, 8:16=z1, 16:24=n1, 24:32=r2,
32:40=z2, 40:48=n2.
"""

import numpy as np
import ml_dtypes

import concourse.bass as bass
import concourse.bacc as bacc
import concourse.tile as tile
from concourse import mybir
from concourse.bass_utils import run_bass_kernel_spmd

F32 = mybir.dt.float32
BF16 = mybir.dt.bfloat16
AF = mybir.ActivationFunctionType
ALU = mybir.AluOpType

V, E, SH, H = 30000, 1024, 2048, 1024
B, T = 64, 50
PAD = 10003
H2 = 2 * H
G = 3 * H2            # 6144 gate rows
NC_N = 8              # cores
BL = B // NC_N        # 8 local batch rows
R = T * BL            # 400 rows per core
KE = E // 128         # 8 k-tiles over E/H
KS = SH // 128        # 16 k-tiles over SH
MG = G // 128         # 48 gate m-tiles
VCH = 1024            # vocab chunk columns per weight tile
N_VCH = (V + VCH - 1) // VCH     # 30 (last = 304)
NB = 512              # psum free chunk

_prog_cache = {}
_PHASE_MARKS = []


def _mark(nc, label):
    n = sum(len(b.instructions) for f in nc.m.functions for b in f.blocks)
    _PHASE_MARKS.append((label, n))


def _build_program(iters=1):
    nc = bacc.Bacc()

    # ---- DRAM parameters (per-core inputs; weights replicated) ----
    embT_d = nc.declare_dram_parameter("embT", [E, R], BF16, isOutput=False)
    sesencT_d = nc.declare_dram_parameter("sesencT", [SH, BL], BF16, isOutput=False)
    lin1WT_d = nc.declare_dram_parameter("lin1WT", [SH, H], BF16, isOutput=False)
    bias_l1_d = nc.declare_dram_parameter("bias_l1", [128, KE], F32, isOutput=False)
    wihT_d = nc.declare_dram_parameter("wihT", [E, G], BF16, isOutput=False)
    bias_i_d = nc.declare_dram_parameter("bias_i", [128, MG], F32, isOutput=False)
    bias_h_d = nc.declare_dram_parameter("bias_h", [128, MG], F32, isOutput=False)
    whhsesT_d = nc.declare_dram_parameter("whhsesT", [H, G], BF16, isOutput=False)
    whh1aT_d = nc.declare_dram_parameter("whh1aT", [H, G // 2], BF16, isOutput=False)
    whh2aT_d = nc.declare_dram_parameter("whh2aT", [H, G // 2], BF16, isOutput=False)
    lin2WT_d = nc.declare_dram_parameter("lin2WT", [H2, E], BF16, isOutput=False)
    bias_l2_d = nc.declare_dram_parameter("bias_l2", [128, KE], F32, isOutput=False)
    outWT_d = nc.declare_dram_parameter("outWT", [E, V], BF16, isOutput=False)

    dec_d = nc.declare_dram_parameter("dec", [R, V], F32, isOutput=True)
    logz_d = nc.declare_dram_parameter("logz", [4, 128], F32, isOutput=True)

    # dram views with partition-dim innermost for DMA
    embT_v = embT_d.rearrange("(k p) r -> p k r", p=128)          # [128, 8, 400]
    sesencT_v = sesencT_d.rearrange("(k p) b -> p k b", p=128)    # [128, 16, 8]
    lin1WT_v = lin1WT_d.rearrange("(k p) h -> p k h", p=128)      # [128, 16, 1024]
    wihT_v = wihT_d.rearrange("(k p) g -> p k g", p=128)          # [128, 8, 6144]
    whhsesT_v = whhsesT_d.rearrange("(k p) g -> p k g", p=128)    # [128, 8, 6144]
    whh1aT_v = whh1aT_d.rearrange("(k p) g -> p k g", p=128)      # [128, 8, 3072]
    whh2aT_v = whh2aT_d.rearrange("(k p) g -> p k g", p=128)
    lin2WT_v = lin2WT_d.rearrange("(k p) e -> p k e", p=128)      # [128, 16, 1024]
    outWT_v = outWT_d.rearrange("(k p) v -> p k v", p=128)        # [128, 8, 30000]

    import contextlib
    with tile.TileContext(nc) as tc:
        with (
            tc.tile_pool(name="const", bufs=1) as cp,
            tc.tile_pool(name="w", bufs=3) as wp,
            tc.tile_pool(name="st", bufs=3) as st,
            tc.tile_pool(name="ps", bufs=6, space="PSUM") as pp,
            tc.For_i(0, iters, 1) if iters > 1 else contextlib.nullcontext(),
        ):
            # ---- persistent tiles ----
            bias_l1 = cp.tile([128, KE], F32)
            nc.sync.dma_start(bias_l1[:], bias_l1_d[:])
            bias_l2 = cp.tile([128, KE], F32)
            nc.sync.dma_start(bias_l2[:], bias_l2_d[:])
            bias_i = cp.tile([128, MG], F32)
            nc.sync.dma_start(bias_i[:], bias_i_d[:])
            bias_h = cp.tile([128, MG], F32)
            nc.sync.dma_start(bias_h[:], bias_h_d[:])
            sesencT = cp.tile([128, KS, BL], BF16)
            nc.sync.dma_start(sesencT[:], sesencT_v[:])
            embT = cp.tile([128, KE, R], BF16)
            nc.sync.dma_start(embT[:], embT_v[:])

            gitT = cp.tile([128, MG, T, BL], BF16)     # gi + biases (+ghses on rz)
            ghsesT = cp.tile([128, MG, BL], F32)       # ses @ whhses.T + bhh
            sesT_f = cp.tile([128, KE, BL], F32)       # tanh'd session enc (f32)
            sesT_b = cp.tile([128, KE, BL], BF16)
            hallT = cp.tile([128, KE, T + 1, BL], BF16)  # [ses, h_1..h_T] (prev-state seq)
            h2T = cp.tile([128, KE, T, BL], BF16)      # second-half h_full
            hidOT = cp.tile([128, KE, R], BF16)
            rowsum = cp.tile([128, 4], F32)
            logZt = cp.tile([128, 4], F32)

            nc.vector.memset(rowsum[:], 0.0)

            _mark(nc, 'start')
            # ---- 1. sesT = tanh(lin1_W @ ses_encoding.T + b) ----
            lin1W_a = wp.tile([128, KE, H], BF16, tag="w")
            nc.sync.dma_start(lin1W_a[:], lin1WT_v[:, 0:KE, :])
            lin1W_b = wp.tile([128, KE, H], BF16, tag="w")
            nc.sync.dma_start(lin1W_b[:], lin1WT_v[:, KE:KS, :])
            ps_ses = pp.tile([128, KE, BL], F32, tag="acc")
            for m in range(KE):
                for k in range(KS):
                    wsrc = lin1W_a if k < KE else lin1W_b
                    nc.tensor.matmul(
                        ps_ses[:, m, :],
                        wsrc[:, k % KE, m * 128:(m + 1) * 128],
                        sesencT[:, k, :],
                        start=(k == 0), stop=(k == KS - 1),
                    )
            for m in range(KE):
                nc.scalar.activation(sesT_f[:, m, :], ps_ses[:, m, :], AF.Tanh,
                                     bias=bias_l1[:, m:m + 1])
            nc.vector.tensor_copy(sesT_b[:], sesT_f[:])
            nc.vector.tensor_copy(hallT[:, :, 0, :], sesT_b[:])

            _mark(nc, 'ses')
            # ---- 2. GI ----
            for c in range(6):
                wih_ch = wp.tile([128, KE, 1024], BF16, tag="w", name=f"wih{c}")
                nc.sync.dma_start(wih_ch[:], wihT_v[:, :, c * 1024:(c + 1) * 1024])
                for mi in range(8):
                    m = c * 8 + mi
                    ps = pp.tile([128, R], F32, tag="acc", name=f"ps_gi{m}")
                    for k in range(KE):
                        nc.tensor.matmul(
                            ps[:],
                            wih_ch[:, k, mi * 128:(mi + 1) * 128],
                            embT[:, k, :],
                            start=(k == 0), stop=(k == KE - 1),
                        )
                    nc.scalar.activation(
                        gitT[:, m, :, :], ps[:], AF.Identity,
                        bias=bias_i[:, m:m + 1])

            _mark(nc, 'gi')
            # ---- 3. ghses ----
            for c in range(6):
                whs_ch = wp.tile([128, KE, 1024], BF16, tag="w", name=f"whs{c}")
                nc.scalar.dma_start(whs_ch[:], whhsesT_v[:, :, c * 1024:(c + 1) * 1024])
                ps = pp.tile([128, 8, BL], F32, tag="acc", name=f"ps_gs{c}")
                for mi in range(8):
                    for k in range(KE):
                        nc.tensor.matmul(
                            ps[:, mi, :],
                            whs_ch[:, k, mi * 128:(mi + 1) * 128],
                            sesT_b[:, k, :],
                            start=(k == 0), stop=(k == KE - 1),
                        )
                for mi in range(8):
                    m = c * 8 + mi
                    nc.scalar.activation(ghsesT[:, m, :], ps[:, mi, :],
                                         AF.Identity, bias=bias_h[:, m:m + 1])

            _mark(nc, 'ghses')
            # ---- 4. fold ghses into gi for the r,z gates (broadcast over t) ----
            nc.vector.tensor_add(
                gitT[:, 0:16, :, :], gitT[:, 0:16, :, :],
                ghsesT[:, 0:16, None, :].broadcast_to([128, 16, T, BL]))
            nc.vector.tensor_add(
                gitT[:, 24:40, :, :], gitT[:, 24:40, :, :],
                ghsesT[:, 24:40, None, :].broadcast_to([128, 16, T, BL]))

            _mark(nc, 'fold')
            # ---- 5. serial GRU recurrence ----
            # whh1a in two 1536-col halves (keeps w-pool slot size at 24KB)
            whh1_h = []
            for h in range(2):
                wt = wp.tile([128, KE, 1536], BF16, tag="w", name=f"whh1_{h}")
                nc.scalar.dma_start(wt[:], whh1aT_v[:, :, h * 1536:(h + 1) * 1536])
                whh1_h.append(wt)

            for t in range(T):
                ps = pp.tile([128, 24, BL], F32, tag="acc", name=f"ps_s{t}")
                for m in range(24):
                    wt = whh1_h[m // 12]
                    mo = (m % 12) * 128
                    for k in range(KE):
                        nc.tensor.matmul(
                            ps[:, m, :],
                            wt[:, k, mo:mo + 128],
                            hallT[:, k, t, :],
                            start=(k == 0), stop=(k == KE - 1),
                        )
                pre = st.tile([128, 16, BL], F32, tag="pre")
                nc.vector.tensor_add(pre[:], ps[:, 0:16, :], gitT[:, 0:16, t, :])
                rz = st.tile([128, 16, BL], F32, tag="rz")
                nc.scalar.activation(rz[:], pre[:], AF.Sigmoid)
                for j in range(8):
                    hn = st.tile([128, BL], F32, tag=f"hn{j % 2}", name=f"hn_{t}_{j}")
                    nc.vector.tensor_add(hn[:], ps[:, 16 + j, :], ghsesT[:, 16 + j, :])
                    nt = st.tile([128, BL], F32, tag=f"nt{j % 2}", name=f"nt_{t}_{j}")
                    nc.vector.tensor_mul(nt[:], rz[:, j, :], hn[:])
                    na = st.tile([128, BL], F32, tag=f"na{j % 2}", name=f"na_{t}_{j}")
                    nc.vector.tensor_add(na[:], nt[:], gitT[:, 16 + j, t, :])
                    nn = st.tile([128, BL], F32, tag=f"nn{j % 2}", name=f"nn_{t}_{j}")
                    nc.scalar.activation(nn[:], na[:], AF.Tanh)
                    dd = st.tile([128, BL], F32, tag=f"dd{j % 2}", name=f"dd_{t}_{j}")
                    nc.vector.tensor_sub(dd[:], hallT[:, j, t, :], nn[:])
                    ee = st.tile([128, BL], F32, tag=f"ee{j % 2}", name=f"ee_{t}_{j}")
                    nc.vector.tensor_mul(ee[:], rz[:, 8 + j, :], dd[:])
                    nc.vector.tensor_add(hallT[:, j, t + 1, :], nn[:], ee[:])

            _mark(nc, 'serial')
            # ---- 6. batched second-half gates ----
            whh2_h = []
            for h in range(2):
                wt = wp.tile([128, KE, 1536], BF16, tag="w", name=f"whh2_{h}")
                nc.sync.dma_start(wt[:], whh2aT_v[:, :, h * 1536:(h + 1) * 1536])
                whh2_h.append(wt)

            TC = T // 2   # t-chunk for elementwise tiles
            for j in range(KE):
                ps_r = pp.tile([128, T, BL], F32, tag="acc", name=f"ps_r2{j}")
                ps_z = pp.tile([128, T, BL], F32, tag="acc", name=f"ps_z2{j}")
                ps_n = pp.tile([128, T, BL], F32, tag="acc", name=f"ps_n2{j}")
                for psx, m in ((ps_r, j), (ps_z, 8 + j), (ps_n, 16 + j)):
                    wt = whh2_h[m // 12]
                    mo = (m % 12) * 128
                    for k in range(KE):
                        nc.tensor.matmul(
                            psx[:],
                            wt[:, k, mo:mo + 128],
                            hallT[:, k, 0:T, :],
                            start=(k == 0), stop=(k == KE - 1),
                        )
                for hh in range(2):
                    ts_ = slice(hh * TC, (hh + 1) * TC)
                    pr2 = st.tile([128, TC, BL], F32, tag="pr2", bufs=2)
                    nc.vector.tensor_add(pr2[:], ps_r[:, ts_, :], gitT[:, 24 + j, ts_, :])
                    r2 = st.tile([128, TC, BL], F32, tag="r2", bufs=2)
                    nc.scalar.activation(r2[:], pr2[:], AF.Sigmoid)
                    pz2 = st.tile([128, TC, BL], F32, tag="pz2", bufs=2)
                    nc.vector.tensor_add(pz2[:], ps_z[:, ts_, :], gitT[:, 32 + j, ts_, :])
                    z2 = st.tile([128, TC, BL], F32, tag="z2", bufs=2)
                    nc.scalar.activation(z2[:], pz2[:], AF.Sigmoid)
                    hn2 = st.tile([128, TC, BL], F32, tag="hn2", bufs=2)
                    nc.vector.tensor_add(
                        hn2[:], ps_n[:, ts_, :],
                        ghsesT[:, 40 + j, None, :].broadcast_to([128, TC, BL]))
                    nt2 = st.tile([128, TC, BL], F32, tag="nt2", bufs=2)
                    nc.vector.tensor_mul(nt2[:], r2[:], hn2[:])
                    na2 = st.tile([128, TC, BL], F32, tag="na2", bufs=2)
                    nc.vector.tensor_add(na2[:], nt2[:], gitT[:, 40 + j, ts_, :])
                    nn2 = st.tile([128, TC, BL], F32, tag="nn2", bufs=2)
                    nc.scalar.activation(nn2[:], na2[:], AF.Tanh)
                    dd2 = st.tile([128, TC, BL], F32, tag="dd2", bufs=2)
                    nc.vector.tensor_sub(
                        dd2[:], sesT_f[:, j, None, :].broadcast_to([128, TC, BL]),
                        nn2[:])
                    ee2 = st.tile([128, TC, BL], F32, tag="ee2", bufs=2)
                    nc.vector.tensor_mul(ee2[:], z2[:], dd2[:])
                    hf2 = st.tile([128, TC, BL], F32, tag="hf2", bufs=2)
                    nc.vector.tensor_add(hf2[:], nn2[:], ee2[:])
                    nc.vector.tensor_copy(h2T[:, j, ts_, :], hf2[:])

            _mark(nc, 'gh2')
            # ---- 7. HidO ----
            lin2W_a = wp.tile([128, KE, E], BF16, tag="w")
            nc.sync.dma_start(lin2W_a[:], lin2WT_v[:, 0:KE, :])
            lin2W_b = wp.tile([128, KE, E], BF16, tag="w")
            nc.sync.dma_start(lin2W_b[:], lin2WT_v[:, KE:2 * KE, :])
            for m in range(KE):
                ps = pp.tile([128, R], F32, tag="acc", name=f"ps_ho{m}")
                for k in range(2 * KE):
                    wsrc = lin2W_a if k < KE else lin2W_b
                    rhs = hallT[:, k, 1:T + 1, :] if k < KE else h2T[:, k - KE, :, :]
                    nc.tensor.matmul(
                        ps[:],
                        wsrc[:, k % KE, m * 128:(m + 1) * 128],
                        rhs,
                        start=(k == 0), stop=(k == 2 * KE - 1),
                    )
                nc.vector.scalar_tensor_tensor(
                    hidOT[:, m, :], ps[:], bias_l2[:, m:m + 1], embT[:, m, :],
                    op0=ALU.add, op1=ALU.add)

            _mark(nc, 'hido')
            # ---- 8. logits + fused exp-sum ----
            m_rows = [128, 128, 128, R - 384]
            for vc in range(N_VCH):
                c0 = vc * VCH
                cw = min(VCH, V - c0)
                wch = wp.tile([128, KE, VCH], BF16, tag="w", name=f"wch{vc}")
                nc.sync.dma_start(wch[:, :, :cw], outWT_v[:, :, c0:c0 + cw])
                for m in range(4):
                    mr = m_rows[m]
                    r0 = m * 128
                    for nb in range((cw + NB - 1) // NB):
                        nw = min(NB, cw - nb * NB)
                        ps = pp.tile([128, NB], F32, tag="acc",
                                     name=f"ps_lg{vc}_{m}_{nb}")
                        for k in range(KE):
                            nc.tensor.matmul(
                                ps[:mr, :nw],
                                hidOT[:, k, r0:r0 + mr],
                                wch[:, k, nb * NB:nb * NB + nw],
                                start=(k == 0), stop=(k == KE - 1),
                            )
                        lg = st.tile([128, NB], F32, tag="lg", bufs=6)
                        nc.vector.tensor_copy(lg[:mr, :nw], ps[:mr, :nw])
                        sc = st.tile([128, NB], F32, tag="sc", bufs=2)
                        sm = st.tile([128, 1], F32, tag="sm", bufs=2)
                        nc.scalar.activation(sc[:mr, :nw], ps[:mr, :nw], AF.Exp,
                                             accum_out=sm[:mr, :])
                        nc.vector.tensor_add(rowsum[:mr, m:m + 1],
                                             rowsum[:mr, m:m + 1], sm[:mr, :])
                        dma_eng = nc.scalar if (m + nb) % 2 else nc.sync
                        dma_eng.dma_start(
                            out=dec_d[r0:r0 + mr, c0 + nb * NB:c0 + nb * NB + nw],
                            in_=lg[:mr, :nw])

            _mark(nc, 'logits')
            # ---- 9. logZ ----
            for m in range(4):
                mr = m_rows[m]
                nc.scalar.activation(logZt[:mr, m:m + 1], rowsum[:mr, m:m + 1], AF.Ln)
                nc.scalar.dma_start(out=logz_d[m, :mr], in_=logZt[:mr, m:m + 1])

    _mark(nc, 'end')
    return nc


def _host_prep(inputs):
    bf = ml_dtypes.bfloat16
    x = np.asarray(inputs["x"])
    emb_W = np.asarray(inputs["emb_W"], np.float32)
    Wih = np.asarray(inputs["gru_Wih"], np.float32)
    Whh = np.asarray(inputs["gru_Whh"], np.float32)
    bih = np.asarray(inputs["gru_bih"], np.float32)
    bhh = np.asarray(inputs["gru_bhh"], np.float32)
    lin1_W = np.asarray(inputs["lin1_W"], np.float32)
    lin1_b = np.asarray(inputs["lin1_b"], np.float32)
    lin2_W = np.asarray(inputs["lin2_W"], np.float32)
    lin2_b = np.asarray(inputs["lin2_b"], np.float32)
    out_W = np.asarray(inputs["out_W"], np.float32)
    ses = np.asarray(inputs["ses_encoding"], np.float32)

    perm = np.r_[0:1024, 2048:3072, 4096:5120, 1024:2048, 3072:4096, 5120:6144]
    shared = {
        "lin1WT": np.ascontiguousarray(lin1_W.T).astype(bf),
        "bias_l1": np.ascontiguousarray(lin1_b.reshape(KE, 128).T).astype(np.float32),
        "wihT": np.ascontiguousarray(Wih[perm].T).astype(bf),
        "bias_i": np.ascontiguousarray(bih[perm].reshape(MG, 128).T).astype(np.float32),
        "bias_h": np.ascontiguousarray(bhh[perm].reshape(MG, 128).T).astype(np.float32),
        "whhsesT": np.ascontiguousarray(Whh[perm, H:].T).astype(bf),
        "whh1aT": np.ascontiguousarray(Whh[perm[:3072], :H].T).astype(bf),
        "whh2aT": np.ascontiguousarray(Whh[perm[3072:], :H].T).astype(bf),
        "lin2WT": np.ascontiguousarray(lin2_W.T).astype(bf),
        "bias_l2": np.ascontiguousarray(lin2_b.reshape(KE, 128).T).astype(np.float32),
        "outWT": np.ascontiguousarray(out_W.T).astype(bf),
    }
    in_maps = []
    for c in range(NC_N):
        bc = slice(c * BL, (c + 1) * BL)
        xe = emb_W[x[bc]]                       # [BL, T, E]
        embT = np.ascontiguousarray(xe.transpose(2, 1, 0).reshape(E, R)).astype(bf)
        sesencT = np.ascontiguousarray(ses[bc].reshape(BL, SH).T).astype(bf)
        in_maps.append({**shared, "embT": embT, "sesencT": sesencT})
    return in_maps, x


def kernel(**inputs):
    if "nc" not in _prog_cache:
        nc = _build_program()
        nc.finalize()          # Bacc.compile: wait-splitting, reg alloc, etc.
        _prog_cache["nc"] = nc
    nc = _prog_cache["nc"]

    in_maps, x = _host_prep(inputs)
    res = run_bass_kernel_spmd(nc, in_maps, core_ids=list(range(NC_N)))

    dec = np.empty((B, T, V), np.float32)
    ll = np.zeros(B, np.float32)
    tok_next = np.concatenate([x[:, 1:], np.zeros((B, 1), x.dtype)], axis=1)
    for c in range(NC_N):
        lg = np.asarray(res.results[c]["dec"]).reshape(T, BL, V)    # [t, b, V]
        dec[c * BL:(c + 1) * BL] = lg.transpose(1, 0, 2)
        logz = np.asarray(res.results[c]["logz"]).reshape(512)[:R].reshape(T, BL)
        for b in range(BL):
            gb = c * BL + b
            tgt = lg[np.arange(T - 1), b, tok_next[gb, :T - 1]]
            ll[gb] = np.sum(tgt - logz[:T - 1, b])
    return dec, ll


# revision 14
# speedup vs baseline: 1.2457x; 1.2457x over previous
"""Trainium2 Bass kernel for the GRU session decoder (nn_Decoder).

Strategy: data-parallel over batch B=64 across 8 NeuronCores (8 rows each).
Each core runs the full pipeline for its batch slice with zero collectives:
  1. ses = tanh(lin1(ses_encoding))            (transposed layouts throughout)
  2. GI = emb @ Wih.T + bih                    (batched over all T)
  3. ghses = ses @ Whh[:, H:].T + bhh          (constant over T)
  4. 50-step GRU recurrence (first-half gates only; weights stream via PE)
  5. second-half gates batched over T -> h_full
  6. HidO = h_full @ lin2.T + lin2_b + emb
  7. logits = HidO @ outW.T  (bf16 matmul, fp32 psum) + fused exp-sum for
     log-softmax denominators
Host side: input transposes/gather, output assembly, tiny target-logit sum.

Row ordering everywhere: r = t*8 + b_local  (T-major), rows per core R=400.
Gate row permutation: [r1, z1, n1, r2, z2, n2] blocks of 1024 rows each,
so in m-tile units (128 rows): 0:8=r1, 8:16=z1, 16:24=n1, 24:32=r2,
32:40=z2, 40:48=n2.
"""

import numpy as np
import ml_dtypes

import concourse.bass as bass
import concourse.bacc as bacc
import concourse.tile as tile
from concourse import mybir
from concourse.bass_utils import run_bass_kernel_spmd

F32 = mybir.dt.float32
BF16 = mybir.dt.bfloat16
AF = mybir.ActivationFunctionType
ALU = mybir.AluOpType

V, E, SH, H = 30000, 1024, 2048, 1024
B, T = 64, 50
PAD = 10003
H2 = 2 * H
G = 3 * H2            # 6144 gate rows
NC_N = 8              # cores
BL = B // NC_N        # 8 local batch rows
R = T * BL            # 400 rows per core
KE = E // 128         # 8 k-tiles over E/H
KS = SH // 128        # 16 k-tiles over SH
MG = G // 128         # 48 gate m-tiles
VCH = 1024            # vocab chunk columns per weight tile
N_VCH = (V + VCH - 1) // VCH     # 30 (last = 304)
NB = 512              # psum free chunk

_prog_cache = {}
_PHASE_MARKS = []


def _mark(nc, label):
    n = sum(len(b.instructions) for f in nc.m.functions for b in f.blocks)
    _PHASE_MARKS.append((label, n))


def _build_program(iters=1):
    nc = bacc.Bacc()

    # ---- DRAM parameters (per-core inputs; weights replicated) ----
    embT_d = nc.declare_dram_parameter("embT", [E, R], BF16, isOutput=False)
    sesencT_d = nc.declare_dram_parameter("sesencT", [SH, BL], BF16, isOutput=False)
    lin1WT_d = nc.declare_dram_parameter("lin1WT", [SH, H], BF16, isOutput=False)
    bias_l1_d = nc.declare_dram_parameter("bias_l1", [128, KE], F32, isOutput=False)
    wihT_d = nc.declare_dram_parameter("wihT", [E, G], BF16, isOutput=False)
    bias_i_d = nc.declare_dram_parameter("bias_i", [128, MG], F32, isOutput=False)
    bias_h_d = nc.declare_dram_parameter("bias_h", [128, MG], F32, isOutput=False)
    whhsesT_d = nc.declare_dram_parameter("whhsesT", [H, G], BF16, isOutput=False)
    whh1aT_d = nc.declare_dram_parameter("whh1aT", [H, G // 2], BF16, isOutput=False)
    whh2aT_d = nc.declare_dram_parameter("whh2aT", [H, G // 2], BF16, isOutput=False)
    lin2WT_d = nc.declare_dram_parameter("lin2WT", [H2, E], BF16, isOutput=False)
    bias_l2_d = nc.declare_dram_parameter("bias_l2", [128, KE], F32, isOutput=False)
    outWT_d = nc.declare_dram_parameter("outWT", [E, V], BF16, isOutput=False)

    dec_d = nc.declare_dram_parameter("dec", [R, V], F32, isOutput=True)
    logz_d = nc.declare_dram_parameter("logz", [4, 128], F32, isOutput=True)

    # dram views with partition-dim innermost for DMA
    embT_v = embT_d.rearrange("(k p) r -> p k r", p=128)          # [128, 8, 400]
    sesencT_v = sesencT_d.rearrange("(k p) b -> p k b", p=128)    # [128, 16, 8]
    lin1WT_v = lin1WT_d.rearrange("(k p) h -> p k h", p=128)      # [128, 16, 1024]
    wihT_v = wihT_d.rearrange("(k p) g -> p k g", p=128)          # [128, 8, 6144]
    whhsesT_v = whhsesT_d.rearrange("(k p) g -> p k g", p=128)    # [128, 8, 6144]
    whh1aT_v = whh1aT_d.rearrange("(k p) g -> p k g", p=128)      # [128, 8, 3072]
    whh2aT_v = whh2aT_d.rearrange("(k p) g -> p k g", p=128)
    lin2WT_v = lin2WT_d.rearrange("(k p) e -> p k e", p=128)      # [128, 16, 1024]
    outWT_v = outWT_d.rearrange("(k p) v -> p k v", p=128)        # [128, 8, 30000]

    import contextlib
    with tile.TileContext(nc) as tc:
        with (
            tc.tile_pool(name="const", bufs=1) as cp,
            tc.tile_pool(name="w", bufs=3) as wp,
            tc.tile_pool(name="st", bufs=3) as st,
            tc.tile_pool(name="ps", bufs=6, space="PSUM") as pp,
            tc.For_i(0, iters, 1) if iters > 1 else contextlib.nullcontext(),
        ):
            # ---- persistent tiles ----
            bias_l1 = cp.tile([128, KE], F32)
            nc.sync.dma_start(bias_l1[:], bias_l1_d[:])
            bias_l2 = cp.tile([128, KE], F32)
            nc.sync.dma_start(bias_l2[:], bias_l2_d[:])
            bias_i = cp.tile([128, MG], F32)
            nc.sync.dma_start(bias_i[:], bias_i_d[:])
            bias_h = cp.tile([128, MG], F32)
            nc.sync.dma_start(bias_h[:], bias_h_d[:])
            sesencT = cp.tile([128, KS, BL], BF16)
            nc.sync.dma_start(sesencT[:], sesencT_v[:])
            embT = cp.tile([128, KE, R], BF16)
            nc.sync.dma_start(embT[:], embT_v[:])

            gitT = cp.tile([128, MG, T, BL], BF16)     # gi + biases (+ghses on rz)
            ghsesT = cp.tile([128, MG, BL], F32)       # ses @ whhses.T + bhh
            sesT_f = cp.tile([128, KE, BL], F32)       # tanh'd session enc (f32)
            sesT_b = cp.tile([128, KE, BL], BF16)
            hallT = cp.tile([128, KE, T + 1, BL], BF16)  # [ses, h_1..h_T] (prev-state seq)
            h2T = cp.tile([128, KE, T, BL], BF16)      # second-half h_full
            hidOT = cp.tile([128, KE, R], BF16)
            rowsum = cp.tile([128, 4], F32)
            logZt = cp.tile([128, 4], F32)

            nc.vector.memset(rowsum[:], 0.0)

            _mark(nc, 'start')
            # ---- 1. sesT = tanh(lin1_W @ ses_encoding.T + b) ----
            lin1W_a = wp.tile([128, KE, H], BF16, tag="w")
            nc.sync.dma_start(lin1W_a[:], lin1WT_v[:, 0:KE, :])
            lin1W_b = wp.tile([128, KE, H], BF16, tag="w")
            nc.sync.dma_start(lin1W_b[:], lin1WT_v[:, KE:KS, :])
            ps_ses = pp.tile([128, KE, BL], F32, tag="acc")
            for m in range(KE):
                for k in range(KS):
                    wsrc = lin1W_a if k < KE else lin1W_b
                    nc.tensor.matmul(
                        ps_ses[:, m, :],
                        wsrc[:, k % KE, m * 128:(m + 1) * 128],
                        sesencT[:, k, :],
                        start=(k == 0), stop=(k == KS - 1),
                    )
            for m in range(KE):
                nc.scalar.activation(sesT_f[:, m, :], ps_ses[:, m, :], AF.Tanh,
                                     bias=bias_l1[:, m:m + 1])
            nc.vector.tensor_copy(sesT_b[:], sesT_f[:])
            nc.vector.tensor_copy(hallT[:, :, 0, :], sesT_b[:])

            _mark(nc, 'ses')
            # ---- 2. GI ----
            for c in range(6):
                wih_ch = wp.tile([128, KE, 1024], BF16, tag="w", name=f"wih{c}")
                nc.sync.dma_start(wih_ch[:], wihT_v[:, :, c * 1024:(c + 1) * 1024])
                for mi in range(8):
                    m = c * 8 + mi
                    ps = pp.tile([128, R], F32, tag="acc", name=f"ps_gi{m}")
                    for k in range(KE):
                        nc.tensor.matmul(
                            ps[:],
                            wih_ch[:, k, mi * 128:(mi + 1) * 128],
                            embT[:, k, :],
                            start=(k == 0), stop=(k == KE - 1),
                        )
                    nc.scalar.activation(
                        gitT[:, m, :, :], ps[:], AF.Identity,
                        bias=bias_i[:, m:m + 1])

            _mark(nc, 'gi')
            # ---- 3. ghses ----
            for c in range(6):
                whs_ch = wp.tile([128, KE, 1024], BF16, tag="w", name=f"whs{c}")
                nc.scalar.dma_start(whs_ch[:], whhsesT_v[:, :, c * 1024:(c + 1) * 1024])
                ps = pp.tile([128, 8, BL], F32, tag="acc", name=f"ps_gs{c}")
                for mi in range(8):
                    for k in range(KE):
                        nc.tensor.matmul(
                            ps[:, mi, :],
                            whs_ch[:, k, mi * 128:(mi + 1) * 128],
                            sesT_b[:, k, :],
                            start=(k == 0), stop=(k == KE - 1),
                        )
                for mi in range(8):
                    m = c * 8 + mi
                    nc.scalar.activation(ghsesT[:, m, :], ps[:, mi, :],
                                         AF.Identity, bias=bias_h[:, m:m + 1])

            _mark(nc, 'ghses')
            # ---- 4. fold ghses into gi for the r,z gates (broadcast over t) ----
            nc.vector.tensor_add(
                gitT[:, 0:16, :, :], gitT[:, 0:16, :, :],
                ghsesT[:, 0:16, None, :].broadcast_to([128, 16, T, BL]))
            nc.vector.tensor_add(
                gitT[:, 24:40, :, :], gitT[:, 24:40, :, :],
                ghsesT[:, 24:40, None, :].broadcast_to([128, 16, T, BL]))

            _mark(nc, 'fold')
            # ---- 5. serial GRU recurrence ----
            # whh1a in two 1536-col halves (keeps w-pool slot size at 24KB)
            whh1_h = []
            for h in range(2):
                wt = wp.tile([128, KE, 1536], BF16, tag="w", name=f"whh1_{h}")
                nc.scalar.dma_start(wt[:], whh1aT_v[:, :, h * 1536:(h + 1) * 1536])
                whh1_h.append(wt)

            for t in range(T):
                ps = pp.tile([128, 24, BL], F32, tag="acc", name=f"ps_s{t}")
                for m in range(24):
                    wt = whh1_h[m // 12]
                    mo = (m % 12) * 128
                    for k in range(KE):
                        nc.tensor.matmul(
                            ps[:, m, :],
                            wt[:, k, mo:mo + 128],
                            hallT[:, k, t, :],
                            start=(k == 0), stop=(k == KE - 1),
                        )
                pre = st.tile([128, 16, BL], F32, tag="pre")
                nc.vector.tensor_add(pre[:], ps[:, 0:16, :], gitT[:, 0:16, t, :])
                rz = st.tile([128, 16, BL], F32, tag="rz")
                nc.scalar.activation(rz[:], pre[:], AF.Sigmoid)
                for j in range(2):
                    jj = slice(16 + 4 * j, 16 + 4 * j + 4)
                    jr = slice(4 * j, 4 * j + 4)
                    jz = slice(8 + 4 * j, 8 + 4 * j + 4)
                    hn = st.tile([128, 4, BL], F32, tag=f"hn{j}", name=f"hn_{t}_{j}")
                    nc.vector.tensor_add(hn[:], ps[:, jj, :], ghsesT[:, jj, :])
                    nt = st.tile([128, 4, BL], F32, tag=f"nt{j}", name=f"nt_{t}_{j}")
                    nc.vector.tensor_mul(nt[:], rz[:, jr, :], hn[:])
                    na = st.tile([128, 4, BL], F32, tag=f"na{j}", name=f"na_{t}_{j}")
                    nc.vector.tensor_add(na[:], nt[:], gitT[:, jj, t, :])
                    nn = st.tile([128, 4, BL], F32, tag=f"nn{j}", name=f"nn_{t}_{j}")
                    nc.scalar.activation(nn[:], na[:], AF.Tanh)
                    dd = st.tile([128, 4, BL], F32, tag=f"dd{j}", name=f"dd_{t}_{j}")
                    nc.vector.tensor_sub(dd[:], hallT[:, 4 * j:4 * j + 4, t, :], nn[:])
                    ee = st.tile([128, 4, BL], F32, tag=f"ee{j}", name=f"ee_{t}_{j}")
                    nc.vector.tensor_mul(ee[:], rz[:, jz, :], dd[:])
                    nc.vector.tensor_add(hallT[:, 4 * j:4 * j + 4, t + 1, :], nn[:], ee[:])

            _mark(nc, 'serial')
            # ---- 6. batched second-half gates ----
            whh2_h = []
            for h in range(2):
                wt = wp.tile([128, KE, 1536], BF16, tag="w", name=f"whh2_{h}")
                nc.sync.dma_start(wt[:], whh2aT_v[:, :, h * 1536:(h + 1) * 1536])
                whh2_h.append(wt)

            TC = T // 2   # t-chunk for elementwise tiles
            for j in range(KE):
                ps_r = pp.tile([128, T, BL], F32, tag="acc", name=f"ps_r2{j}")
                ps_z = pp.tile([128, T, BL], F32, tag="acc", name=f"ps_z2{j}")
                ps_n = pp.tile([128, T, BL], F32, tag="acc", name=f"ps_n2{j}")
                for psx, m in ((ps_r, j), (ps_z, 8 + j), (ps_n, 16 + j)):
                    wt = whh2_h[m // 12]
                    mo = (m % 12) * 128
                    for k in range(KE):
                        nc.tensor.matmul(
                            psx[:],
                            wt[:, k, mo:mo + 128],
                            hallT[:, k, 0:T, :],
                            start=(k == 0), stop=(k == KE - 1),
                        )
                for hh in range(2):
                    ts_ = slice(hh * TC, (hh + 1) * TC)
                    pr2 = st.tile([128, TC, BL], F32, tag="pr2", bufs=2)
                    nc.vector.tensor_add(pr2[:], ps_r[:, ts_, :], gitT[:, 24 + j, ts_, :])
                    r2 = st.tile([128, TC, BL], F32, tag="r2", bufs=2)
                    nc.scalar.activation(r2[:], pr2[:], AF.Sigmoid)
                    pz2 = st.tile([128, TC, BL], F32, tag="pz2", bufs=2)
                    nc.vector.tensor_add(pz2[:], ps_z[:, ts_, :], gitT[:, 32 + j, ts_, :])
                    z2 = st.tile([128, TC, BL], F32, tag="z2", bufs=2)
                    nc.scalar.activation(z2[:], pz2[:], AF.Sigmoid)
                    hn2 = st.tile([128, TC, BL], F32, tag="hn2", bufs=2)
                    nc.vector.tensor_add(
                        hn2[:], ps_n[:, ts_, :],
                        ghsesT[:, 40 + j, None, :].broadcast_to([128, TC, BL]))
                    nt2 = st.tile([128, TC, BL], F32, tag="nt2", bufs=2)
                    nc.vector.tensor_mul(nt2[:], r2[:], hn2[:])
                    na2 = st.tile([128, TC, BL], F32, tag="na2", bufs=2)
                    nc.vector.tensor_add(na2[:], nt2[:], gitT[:, 40 + j, ts_, :])
                    nn2 = st.tile([128, TC, BL], F32, tag="nn2", bufs=2)
                    nc.scalar.activation(nn2[:], na2[:], AF.Tanh)
                    dd2 = st.tile([128, TC, BL], F32, tag="dd2", bufs=2)
                    nc.vector.tensor_sub(
                        dd2[:], sesT_f[:, j, None, :].broadcast_to([128, TC, BL]),
                        nn2[:])
                    ee2 = st.tile([128, TC, BL], F32, tag="ee2", bufs=2)
                    nc.vector.tensor_mul(ee2[:], z2[:], dd2[:])
                    hf2 = st.tile([128, TC, BL], F32, tag="hf2", bufs=2)
                    nc.vector.tensor_add(hf2[:], nn2[:], ee2[:])
                    nc.vector.tensor_copy(h2T[:, j, ts_, :], hf2[:])

            _mark(nc, 'gh2')
            # ---- 7. HidO ----
            lin2W_a = wp.tile([128, KE, E], BF16, tag="w")
            nc.sync.dma_start(lin2W_a[:], lin2WT_v[:, 0:KE, :])
            lin2W_b = wp.tile([128, KE, E], BF16, tag="w")
            nc.sync.dma_start(lin2W_b[:], lin2WT_v[:, KE:2 * KE, :])
            for m in range(KE):
                ps = pp.tile([128, R], F32, tag="acc", name=f"ps_ho{m}")
                for k in range(2 * KE):
                    wsrc = lin2W_a if k < KE else lin2W_b
                    rhs = hallT[:, k, 1:T + 1, :] if k < KE else h2T[:, k - KE, :, :]
                    nc.tensor.matmul(
                        ps[:],
                        wsrc[:, k % KE, m * 128:(m + 1) * 128],
                        rhs,
                        start=(k == 0), stop=(k == 2 * KE - 1),
                    )
                nc.vector.scalar_tensor_tensor(
                    hidOT[:, m, :], ps[:], bias_l2[:, m:m + 1], embT[:, m, :],
                    op0=ALU.add, op1=ALU.add)

            _mark(nc, 'hido')
            # ---- 8. logits + fused exp-sum ----
            m_rows = [128, 128, 128, R - 384]
            for vc in range(N_VCH):
                c0 = vc * VCH
                cw = min(VCH, V - c0)
                wch = wp.tile([128, KE, VCH], BF16, tag="w", name=f"wch{vc}")
                nc.sync.dma_start(wch[:, :, :cw], outWT_v[:, :, c0:c0 + cw])
                for m in range(4):
                    mr = m_rows[m]
                    r0 = m * 128
                    for nb in range((cw + NB - 1) // NB):
                        nw = min(NB, cw - nb * NB)
                        ps = pp.tile([128, NB], F32, tag="acc",
                                     name=f"ps_lg{vc}_{m}_{nb}")
                        for k in range(KE):
                            nc.tensor.matmul(
                                ps[:mr, :nw],
                                hidOT[:, k, r0:r0 + mr],
                                wch[:, k, nb * NB:nb * NB + nw],
                                start=(k == 0), stop=(k == KE - 1),
                            )
                        lg = st.tile([128, NB], F32, tag="lg", bufs=6)
                        nc.vector.tensor_copy(lg[:mr, :nw], ps[:mr, :nw])
                        sc = st.tile([128, NB], F32, tag="sc", bufs=2)
                        sm = st.tile([128, 1], F32, tag="sm", bufs=2)
                        nc.scalar.activation(sc[:mr, :nw], ps[:mr, :nw], AF.Exp,
                                             accum_out=sm[:mr, :])
                        nc.vector.tensor_add(rowsum[:mr, m:m + 1],
                                             rowsum[:mr, m:m + 1], sm[:mr, :])
                        dma_eng = nc.scalar if (m + nb) % 2 else nc.sync
                        dma_eng.dma_start(
                            out=dec_d[r0:r0 + mr, c0 + nb * NB:c0 + nb * NB + nw],
                            in_=lg[:mr, :nw])

            _mark(nc, 'logits')
            # ---- 9. logZ ----
            for m in range(4):
                mr = m_rows[m]
                nc.scalar.activation(logZt[:mr, m:m + 1], rowsum[:mr, m:m + 1], AF.Ln)
                nc.scalar.dma_start(out=logz_d[m, :mr], in_=logZt[:mr, m:m + 1])

    _mark(nc, 'end')
    return nc


def _host_prep(inputs):
    bf = ml_dtypes.bfloat16
    x = np.asarray(inputs["x"])
    emb_W = np.asarray(inputs["emb_W"], np.float32)
    Wih = np.asarray(inputs["gru_Wih"], np.float32)
    Whh = np.asarray(inputs["gru_Whh"], np.float32)
    bih = np.asarray(inputs["gru_bih"], np.float32)
    bhh = np.asarray(inputs["gru_bhh"], np.float32)
    lin1_W = np.asarray(inputs["lin1_W"], np.float32)
    lin1_b = np.asarray(inputs["lin1_b"], np.float32)
    lin2_W = np.asarray(inputs["lin2_W"], np.float32)
    lin2_b = np.asarray(inputs["lin2_b"], np.float32)
    out_W = np.asarray(inputs["out_W"], np.float32)
    ses = np.asarray(inputs["ses_encoding"], np.float32)

    perm = np.r_[0:1024, 2048:3072, 4096:5120, 1024:2048, 3072:4096, 5120:6144]
    shared = {
        "lin1WT": np.ascontiguousarray(lin1_W.T).astype(bf),
        "bias_l1": np.ascontiguousarray(lin1_b.reshape(KE, 128).T).astype(np.float32),
        "wihT": np.ascontiguousarray(Wih[perm].T).astype(bf),
        "bias_i": np.ascontiguousarray(bih[perm].reshape(MG, 128).T).astype(np.float32),
        "bias_h": np.ascontiguousarray(bhh[perm].reshape(MG, 128).T).astype(np.float32),
        "whhsesT": np.ascontiguousarray(Whh[perm, H:].T).astype(bf),
        "whh1aT": np.ascontiguousarray(Whh[perm[:3072], :H].T).astype(bf),
        "whh2aT": np.ascontiguousarray(Whh[perm[3072:], :H].T).astype(bf),
        "lin2WT": np.ascontiguousarray(lin2_W.T).astype(bf),
        "bias_l2": np.ascontiguousarray(lin2_b.reshape(KE, 128).T).astype(np.float32),
        "outWT": np.ascontiguousarray(out_W.T).astype(bf),
    }
    in_maps = []
    for c in range(NC_N):
        bc = slice(c * BL, (c + 1) * BL)
        xe = emb_W[x[bc]]                       # [BL, T, E]
        embT = np.ascontiguousarray(xe.transpose(2, 1, 0).reshape(E, R)).astype(bf)
        sesencT = np.ascontiguousarray(ses[bc].reshape(BL, SH).T).astype(bf)
        in_maps.append({**shared, "embT": embT, "sesencT": sesencT})
    return in_maps, x


def kernel(**inputs):
    if "nc" not in _prog_cache:
        nc = _build_program()
        nc.finalize()          # Bacc.compile: wait-splitting, reg alloc, etc.
        _prog_cache["nc"] = nc
    nc = _prog_cache["nc"]

    in_maps, x = _host_prep(inputs)
    res = run_bass_kernel_spmd(nc, in_maps, core_ids=list(range(NC_N)))

    dec = np.empty((B, T, V), np.float32)
    ll = np.zeros(B, np.float32)
    tok_next = np.concatenate([x[:, 1:], np.zeros((B, 1), x.dtype)], axis=1)
    for c in range(NC_N):
        lg = np.asarray(res.results[c]["dec"]).reshape(T, BL, V)    # [t, b, V]
        dec[c * BL:(c + 1) * BL] = lg.transpose(1, 0, 2)
        logz = np.asarray(res.results[c]["logz"]).reshape(512)[:R].reshape(T, BL)
        for b in range(BL):
            gb = c * BL + b
            tgt = lg[np.arange(T - 1), b, tok_next[gb, :T - 1]]
            ll[gb] = np.sum(tgt - logz[:T - 1, b])
    return dec, ll
